# revision 64
# baseline (speedup 1.0000x reference)
"""BERT-CRF loss kernel for Trainium2 (8 NeuronCores, data-parallel over positions).

Math: loss = sum_b(forward_b - cumsum(gold)_b) for a CRF whose forward scan runs
over the flattened B*S steps (batch carryover).  The log-semiring scan is
reassociated into per-chunk (L=2 positions) transfer matrices computed on
device in scaled probability space:

  feats[pos,t] = hidden @ W.T + b     (PE, fp8, pos-major)
  m[pos]       = max over live tags   (DVE reduce from PSUM)
  fsub         = feats - m            (DVE sub, bf16, shipped for gold score)
  EF           = exp(fsub)            (ACT, bf16, pair-packed; PE transposes
                                       to tag-major PSUM)
  chunk E:  A = diag(EF_1) Elive diag(EF_0) Elive
            At0 <- e40jm * EF_0 ; ps <- Eblk.T @ At0 ; A <- ps * EF_1
            (one DVE mul + one PE matmul + one DVE mul per chain; EF read
             directly from bf16 PSUM at DVE 2x)

Positions are column-permuted on the host so that each chain step reads a
contiguous 32-column EF slice, and the 8 sentences (groups) are pair-packed
at partition offsets {0,16} inside 32-aligned slots (PE transposes write
[32,128] blocks at legal partition starts).

Every engine instruction and DMA may carry at most ONE semaphore wait on this
target, so multi-dep points are preceded by tiny data-dependent "absorber"
instructions that pull foreign sems into the local engine clock.

Host combines the 16384 chunk matrices (f64, vectorized pairwise tree per
sentence + sequential sentence carry) and computes the gold score from the
shipped fsub + m.
"""
import numpy as np
import ml_dtypes
from contextlib import ExitStack

import concourse.bass as bass
import concourse.mybir as mybir
from concourse.tile import TileContext
from concourse.tile_rust import add_dep_helper
from concourse.bass_utils import run_bass_kernel_spmd

B, S, H, T = 64, 512, 768, 12
START, STOP, NEG = 10, 11, -10000.0
L = 2                      # chunk length (positions per transfer matrix)
NCORES = 8
P_CORE = B * S // NCORES   # 4096 positions per core
G = 8                      # sentences (groups) per core
KPG = S // L               # 256 chunks per group
NE = 8                     # scan chains per core
CPE = KPG // NE            # 32 chunks per group per chain
NLIVE = 10

FP8NP = ml_dtypes.float8_e4m3fn
BF16 = ml_dtypes.bfloat16

F32 = mybir.dt.float32
BF = mybir.dt.bfloat16
FP8 = mybir.dt.float8e4

# device column <-> original position permutation (per core)
# position q = g*512 + k*2 + s ; chain E = k//32 ; c = k%32
# col = (E//2)*1024 + g*128 + (E%2)*64 + s*32 + c
_cols = np.arange(P_CORE)
_P = _cols // 1024                   # quarter (chain pair)
_g = (_cols % 1024) // 128
_in = _cols % 128
_E = _P * 2 + _in // 64
_s = (_in % 64) // 32
_c = _in % 32
PERM = (_g * S + (_E * CPE + _c) * L + _s)   # PERM[col] = original position


def _build_nc():
    nc = bass.Bass()
    hidT = nc.declare_dram_parameter("hidT", [H, P_CORE], FP8, isOutput=False)
    cf8 = nc.declare_dram_parameter("cf8", [128, 212], FP8, isOutput=False)
    cb16 = nc.declare_dram_parameter("cb16", [128, 576], BF, isOutput=False)
    fm_out = nc.declare_dram_parameter("fm_out", [128, 416], BF, isOutput=True)
    a_out = nc.declare_dram_parameter("a_out", [128, NE * 320], BF, isOutput=True)

    with ExitStack() as ctx:
        tc = ctx.enter_context(TileContext(nc))
        const_pool = ctx.enter_context(tc.tile_pool(name="const", bufs=1))
        hid_pool = ctx.enter_context(tc.tile_pool(name="hid", bufs=12))
        sb_pool = ctx.enter_context(tc.tile_pool(name="sb", bufs=1))
        at_pool = ctx.enter_context(tc.tile_pool(name="at", bufs=4))
        fp_pool = ctx.enter_context(tc.tile_pool(name="fps", bufs=1, space="PSUM"))
        eft_pool = ctx.enter_context(tc.tile_pool(name="eftp", bufs=1, space="PSUM"))
        ps_pool = ctx.enter_context(tc.tile_pool(name="pss", bufs=2, space="PSUM"))
        warm_pool = ctx.enter_context(tc.tile_pool(name="warm", bufs=1, space="PSUM"))

        # ---- persistent SBUF tiles ----
        cf8_sb = const_pool.tile([128, 212], FP8)
        cb16_sb = const_pool.tile([128, 576], BF)
        efpos = sb_pool.tile([128, 512], BF)       # pos-major EF, pair-packed
        fraws = [sb_pool.tile([128, 96], BF, name=f"fraw{q}")
                 for q in range(4)]                # feats psum copied to SBUF
        fm_sb = sb_pool.tile([128, 416], BF)       # m (cols 0:32) | fsub (32:416)
        a_sb = sb_pool.tile([128, NE * 320], BF)   # final chunk matrices
        efsbs = [sb_pool.tile([128, 128], BF, name=f"efsb{q}")
                 for q in range(4)]                # tag-major EF (SBUF copy)
        scrqs = [sb_pool.tile([1, 2], BF, name=f"scrq{q}")
                 for q in range(4)]                # per-quarter absorber scratch
        scrps = [sb_pool.tile([1, 2], BF, name=f"scrp{q}")
                 for q in range(4)]                # per-quarter ACT scratch
        scrbs = [sb_pool.tile([1, 2], BF, name=f"scrb{q}")
                 for q in range(4)]                # per-quarter Pool-sub scratch

        eft_ps = eft_pool.tile([128, 512], BF)     # transposed EF psum

        all_dmas = []
        out_dmas = []
        last_insts = {}
        hid = {}

        def hdma(eng, hs, h):
            t = hid_pool.tile([128, 2048], FP8, name=f"hid_{hs}_{h}", tag="hid")
            di = eng.dma_start(
                out=t[:, :],
                in_=hidT[hs * 128:(hs + 1) * 128, h * 2048:(h + 1) * 2048])
            all_dmas.append(di)
            hid[(hs, h)] = t

        # ---- pad init on DVE: zero the 4-wide pad cols of efpos ----
        memset_i = nc.vector.memset(
            bass.AP(efpos.tensor, efpos[:, 12:16].offset,
                    [efpos[:, :].ap[0], [16, 32], [1, 4]]),
            0.0,
        )

        # ---- consts on ACT; hid on Pool/SP ----
        all_dmas.append(nc.scalar.dma_start(out=cf8_sb[:, :], in_=cf8[:, :]))
        all_dmas.append(nc.scalar.dma_start(out=cb16_sb[:, :], in_=cb16[:, :]))
        hdma(nc.gpsimd, 0, 0)
        hdma(nc.sync, 3, 0)
        hdma(nc.gpsimd, 1, 0)
        hdma(nc.sync, 4, 0)
        hdma(nc.gpsimd, 2, 0)
        hdma(nc.sync, 5, 0)

        ident = cb16_sb[:, 0:128]
        eblk = cb16_sb[:, 128:256]
        e40jm = cb16_sb[:, 256:576]

        # ---- warmups: ramp PE, load ACT exp table, absorb const/memset sems
        wp = warm_pool.tile([128, 384], F32)
        scr = const_pool.tile([1, 16], BF)
        nc.scalar.activation(scr[0:1, 0:8], cf8_sb[0:1, 0:8],
                             mybir.ActivationFunctionType.Exp)
        for _ in range(3):
            nc.tensor.matmul(wp[:, 0:212], lhsT=cf8_sb[:, 0:128],
                             rhs=cf8_sb[:, 0:212], start=True, stop=True)
        wm = nc.tensor.matmul(wp[:, 0:128], lhsT=cb16_sb[:, 0:128],
                              rhs=cb16_sb[:, 0:128], start=True, stop=True)
        add_dep_helper(wm.ins, memset_i.ins, True, "absorb memset sem into PE")
        scrp = const_pool.tile([1, 16], BF)
        # absorb the cb16 DMA-queue sem into the DVE clock (for e40jm/s0 muls)
        nc.vector.tensor_copy(scrp[0:1, 8:10], cb16_sb[0:1, 256:258])

        fps = {}

        def feats_mms(Q):
            h = Q // 2
            fp_ps = fp_pool.tile([128, 96], F32, name=f"fp{Q}", tag="fp")
            fps[Q] = fp_ps
            if Q >= 1:
                # absorber pair for the recycled feats-psum bank
                ab0 = nc.tensor.matmul(wp[0:1, 0:1], lhsT=cf8_sb[0:1, 0:1],
                                       rhs=fraws[Q - 1][0:1, 0:1],
                                       start=True, stop=True)
                ab1 = nc.tensor.matmul(fp_ps[0:1, 0:1], lhsT=cf8_sb[0:1, 0:1],
                                       rhs=cf8_sb[0:1, 0:1], start=True,
                                       stop=True, skip_group_check=True)
                add_dep_helper(ab1.ins, ab0.ins, False, "absorber ordering")
            for g in range(G):
                blk = fp_ps[:, g * 12:(g + 1) * 12]
                for hs in range(6):
                    nc.tensor.matmul(
                        blk,
                        lhsT=hid[(hs, h)][:, (Q % 2) * 1024 + g * 128:
                                          (Q % 2) * 1024 + (g + 1) * 128],
                        rhs=cf8_sb[:, hs * 12:(hs + 1) * 12],
                        start=(hs == 0), stop=False,
                        skip_group_check=True,
                    )
                nc.tensor.matmul(
                    blk, lhsT=cf8_sb[0:1, 72:200], rhs=cf8_sb[0:1, 200:212],
                    start=False, stop=True, skip_group_check=True,
                )

        def feats_tail(Q):
            fp_ps = fps[Q]
            fraw = fraws[Q]
            # copy feats psum -> SBUF bf16 (ACT)
            cp_i = nc.scalar.activation(
                fraw[:, :], fp_ps[:, :], mybir.ActivationFunctionType.Copy)
            last_insts['actcp'] = cp_i
            # m = max over live tags (DVE, bf16 2x)
            fq3 = fraw[:, :].rearrange("p (b j) -> p b j", j=12)
            nc.vector.reduce_max(
                out=fm_sb[:, Q * 8:(Q + 1) * 8],
                in_=bass.AP(fq3.tensor, fq3.offset,
                            [fq3.ap[0], fq3.ap[1], [1, NLIVE]]),
                axis=mybir.AxisListType.X,
            )
            # fsub = feats - m (DVE, all SBUF bf16)
            msl = fm_sb[:, Q * 8:(Q + 1) * 8]
            m_b = bass.AP(msl.tensor, msl.offset,
                          [msl.ap[0], msl.ap[1], [0, 12]])
            if Q >= 2:
                # B-half subs run on Pool (idle after its DMAs) so they do
                # not contend with the chain muls on DVE.  Absorber pulls the
                # DVE (reduce) sem into the Pool clock first.
                nc.gpsimd.tensor_copy(scrbs[Q][0:1, 0:2],
                                      fm_sb[0:1, Q * 8:Q * 8 + 2])
                last_insts['pool'] = nc.gpsimd.tensor_sub(
                    fm_sb[:, 32 + Q * 96:32 + (Q + 1) * 96]
                    .rearrange("p (b j) -> p b j", j=12),
                    fq3, m_b,
                )
            else:
                last_insts['dve'] = nc.vector.tensor_sub(
                    fm_sb[:, 32 + Q * 96:32 + (Q + 1) * 96]
                    .rearrange("p (b j) -> p b j", j=12),
                    fq3, m_b,
                )
            # EF = exp(fsub) into pair-packed layout (ACT)
            eo = efpos[:, Q * 128:(Q + 1) * 128]
            eo3 = bass.AP(eo.tensor, eo.offset, [eo.ap[0], [16, 8], [1, 12]])
            last_insts['act'] = nc.scalar.activation(
                eo3,
                fm_sb[:, 32 + Q * 96:32 + (Q + 1) * 96]
                .rearrange("p (b j) -> p b j", j=12),
                mybir.ActivationFunctionType.Exp,
            )

        def transposes(Q):
            for v in range(4):
                nc.tensor.transpose(
                    eft_ps[32 * v:32 * v + 32, Q * 128:(Q + 1) * 128],
                    efpos[:, Q * 128 + v * 32:Q * 128 + v * 32 + 32],
                    ident,
                    tile_position=(0, 32 * v),
                )
            # s=1 EF slices must live in SBUF: the s1 mul already reads the
            # matmul output from PSUM and HW allows only one PSUM input.
            # (tiny absorber first: pulls the PE transpose sem into the DVE
            # clock so the copy itself needs at most one wait)
            nc.vector.tensor_copy(scrqs[Q][0:1, 0:2],
                                  eft_ps[0:1, Q * 128 + 32:Q * 128 + 34])
            nc.vector.tensor_copy(ef1s[Q][:, :],
                                  eft_ps[:, Q * 128 + 32:Q * 128 + 128])

        def ef_ap(E, s):
            if s == 0:
                col = (E // 2) * 128 + (E % 2) * 64
                base = eft_ps[:, col:col + 32]
            else:
                col = (E % 2) * 64
                base = ef1s[E // 2][:, col:col + 32]
            return bass.AP(base.tensor, base.offset,
                           [base.ap[0], [0, NLIVE], base.ap[1]])

        ats = {}

        def chain_s0_pair(Q):
            # one DVE op computes both chains' s0 for the quarter:
            # at2[p, j*64 + e*32 + c] = e40jm[p, j*32+c] * EF[p, e*64+c]
            at2 = at_pool.tile([128, 640], BF, name=f"at_{Q}", tag="at")
            ef = efsbs[Q]
            nc.vector.tensor_mul(
                bass.AP(at2.tensor, at2[:, :].offset,
                        [at2[:, :].ap[0], [32, NLIVE], [320, 2], [1, 32]]),
                bass.AP(cb16_sb.tensor, e40jm.offset,
                        [e40jm.ap[0], [32, NLIVE], [0, 2], [1, 32]]),
                bass.AP(ef.tensor, ef[:, :].offset,
                        [ef[:, :].ap[0], [0, NLIVE], [64, 2], [1, 32]]),
            )
            ats[2 * Q] = at2
            ats[2 * Q + 1] = at2

        ps6_sb = sb_pool.tile([128, 320], BF)      # E6 ps staged via ACT
        pmuls = {}

        def chain_s1(E):
            at2 = ats[E]
            at = at2[:, (E % 2) * 320:(E % 2) * 320 + 320]
            # absorber 1: pull the DVE (At ready) sem into the PE clock
            ab1 = nc.tensor.matmul(wp[0:1, 0:1], lhsT=cb16_sb[0:1, 0:1],
                                   rhs=at[0:1, 0:1], start=True, stop=True)
            ps = ps_pool.tile([128, 320], F32)
            # absorber 2: dummy first-writer carries the PSUM bank hazard
            ab2 = nc.tensor.matmul(ps[0:1, 0:1], lhsT=cb16_sb[0:1, 0:1],
                                   rhs=cb16_sb[0:1, 0:1], start=True,
                                   stop=True, skip_group_check=True)
            add_dep_helper(ab2.ins, ab1.ins, False, "absorber ordering")
            last_insts['pe'] = nc.tensor.matmul(
                ps[:, :], lhsT=eblk, rhs=at,
                start=True, stop=True, skip_group_check=True)
            if E == 6:
                # late-window offload: ACT stages ps to SBUF, Pool does the
                # mul - takes one 458ns mul off the saturated DVE tail
                last_insts['actps'] = nc.scalar.activation(
                    ps6_sb[:, :], ps[:, :], mybir.ActivationFunctionType.Copy)
                pmuls['e6'] = nc.gpsimd.tensor_mul(
                    a_sb[:, E * 320:(E + 1) * 320]
                    .rearrange("p (j c) -> p j c", c=32),
                    ps6_sb[:, :].rearrange("p (j c) -> p j c", c=32),
                    ef_ap(E, 1),
                )
                last_insts['pool'] = pmuls['e6']
            else:
                last_insts['dve'] = nc.vector.tensor_mul(
                    a_sb[:, E * 320:(E + 1) * 320]
                    .rearrange("p (j c) -> p j c", c=32),
                    ps[:, :].rearrange("p (j c) -> p j c", c=32),
                    ef_ap(E, 1),
                )

        def aout(lo, hi):
            # Pool absorber pulls the (DVE) producer sem in, then the SWDGE
            # DMA needs only its queue wait
            if lo == 1920 and 'e6' in pmuls:
                # also pre-absorb the Pool-self sem of the offloaded E6 mul
                # (SWDGE transfers do not respect engine order)
                abp = nc.gpsimd.tensor_copy(scrp[0:1, 12:14],
                                            scrbs[2][0:1, 0:2])
                add_dep_helper(abp.ins, pmuls['e6'].ins, True,
                               "absorb E6 pool mul for aout")
            last_insts['poolab'] = nc.gpsimd.tensor_copy(
                scrp[0:1, (lo // 640) * 2:(lo // 640) * 2 + 2],
                a_sb[0:1, hi - 2:hi])
            out_dmas.append(nc.gpsimd.dma_start(out=a_out[:, lo:hi],
                                                in_=a_sb[:, lo:hi]))

        # ================= emission order =================
        feats_mms(0)
        feats_mms(1)
        hdma(nc.gpsimd, 0, 1)
        hdma(nc.sync, 3, 1)
        feats_tail(0)
        hdma(nc.gpsimd, 1, 1)
        hdma(nc.sync, 4, 1)
        feats_tail(1)
        hdma(nc.gpsimd, 2, 1)
        hdma(nc.sync, 5, 1)
        transposes(0)
        transposes(1)
        chain_s0_pair(0)
        chain_s0_pair(1)
        chain_s1(0)
        feats_mms(2)
        feats_tail(2)
        chain_s1(1)
        feats_mms(3)
        feats_tail(3)
        chain_s1(2)
        chain_s1(3)
        aout(0, 1280)
        # fm_out after the last sub (Q3)
        nc.gpsimd.tensor_copy(scrp[0:1, 14:16], fm_sb[0:1, 414:416])
        out_dmas.append(nc.gpsimd.dma_start(out=fm_out[:, :], in_=fm_sb[:, :]))
        transposes(2)
        transposes(3)
        chain_s0_pair(2)
        chain_s0_pair(3)
        chain_s1(4)
        chain_s1(5)
        aout(1280, 1920)
        chain_s1(6)
        chain_s1(7)
        aout(1920, 2560)
        # Pre-absorb every proc's clock into SP one dep at a time, so the
        # Tile tail drain does not need a multi-sem wait.
        for dep in all_dmas + list(last_insts.values()) + out_dmas:
            nop = nc.sync.nop()
            add_dep_helper(nop.ins, dep.ins, True, "drain preclear")
    return nc


_NC_CACHE = None


def _get_nc():
    global _NC_CACHE
    if _NC_CACHE is None:
        _NC_CACHE = _build_nc()
    return _NC_CACHE


def _build_consts(W, b, transitions):
    E = np.exp(transitions.astype(np.float64))
    E[START, :] = 0.0
    E[STOP, :] = 0.0
    E[:, STOP] = 0.0
    E = E.astype(np.float32)

    cf8 = np.zeros((128, 212), np.float32)
    cf8[:, 0:72] = W.T.reshape(6, 128, T).transpose(1, 0, 2).reshape(128, 72)
    cf8[0, 72:200] = 1.0
    cf8[0, 200:212] = b
    cf8 = cf8.astype(FP8NP)

    cb16 = np.zeros((128, 576), np.float32)
    cb16[:, 0:128] = np.eye(128)
    # eblk: eblk[32v+off+j, 32v+off+i] = E[i, j]  (live 10x10)
    for v in range(4):
        for off in (0, 16):
            o = 32 * v + off
            cb16[o:o + NLIVE, 128 + o:128 + o + NLIVE] = E[:NLIVE, :NLIVE].T
    # e40jm: e40jm[32v+off+i, j*32+c] = E[i, j]
    blk = np.zeros((32, 320), np.float32)
    for off in (0, 16):
        for i in range(NLIVE):
            for j in range(NLIVE):
                blk[off + i, j * 32:(j + 1) * 32] = E[i, j]
    for v in range(4):
        cb16[32 * v:32 * v + 32, 256:576] = blk
    cb16 = cb16.astype(BF16)
    return cf8, cb16


def _run_device(hidden, W, b, transitions, trace=False, tmpdir=None):
    cf8, cb16 = _build_consts(W, b, transitions)
    flat = hidden.reshape(B * S, H)
    in_maps = []
    for core in range(NCORES):
        blk = flat[core * P_CORE:(core + 1) * P_CORE]        # [4096, 768]
        hT = np.ascontiguousarray(blk[PERM].T).astype(FP8NP)  # [768, 4096]
        in_maps.append({"hidT": hT, "cf8": cf8, "cb16": cb16})
    return run_bass_kernel_spmd(
        _get_nc(), in_maps, list(range(NCORES)), trace=trace, tmpdir=tmpdir)


def _logsumexp(x, axis):
    mx = np.max(x, axis=axis)
    mx_safe = np.where(np.isfinite(mx), mx, 0.0)
    out = mx + np.log(np.sum(np.exp(x - np.expand_dims(mx_safe, axis)), axis=axis))
    return np.where(np.isfinite(mx), out, -np.inf)


def _host_combine(results, transitions, tags):
    trans = transitions.astype(np.float64)
    err = np.errstate(invalid="ignore", divide="ignore", over="ignore")
    err.__enter__()

    # unpack fm_out: m and fsub in device (permuted) order -> original order
    feats = np.zeros((NCORES, P_CORE, T), np.float64)
    m_all = np.zeros((NCORES, P_CORE), np.float64)
    tt = np.arange(32)
    pp = np.arange(128)
    colidx = (tt[None, :] // 8) * 1024 + (tt[None, :] % 8) * 128 + pp[:, None]
    q_of = PERM[colidx]                # [p, t] original position
    for core, r in enumerate(results):
        fm = np.asarray(r["fm_out"]).astype(np.float64)   # [128, 416]
        m_dev = fm[:, 0:32]            # [p, t]
        fs_dev = fm[:, 32:416].reshape(128, 32, 12)   # [p, t, j]
        m_all[core, q_of] = m_dev
        feats[core, q_of, :] = fs_dev + m_dev[:, :, None]

    feats = feats.reshape(B, S, T)
    m_all = m_all.reshape(B, S)

    # unpack chunk matrices: A[b, k][i, j] (live 10x10), log + scale
    logA = np.zeros((B, KPG, NLIVE, NLIVE), np.float64)
    for core, r in enumerate(results):
        a = np.asarray(r["a_out"]).astype(np.float64)     # [128, 2560]
        a4 = a.reshape(128, NE, NLIVE, CPE)                # [p, E, j, c]
        for g in range(G):
            rows = 32 * (g // 2) + 16 * (g % 2)
            blkm = a4[rows:rows + NLIVE]                   # [i, E, j, c]
            logA[core * G + g] = np.log(blkm).transpose(1, 3, 0, 2).reshape(
                KPG, NLIVE, NLIVE)
    scale = m_all.reshape(B, KPG, L).sum(axis=2)           # [B, KPG]
    logA = logA + scale[:, :, None, None]

    # first global chunk: explicit recurrence from init (full 12-state)
    v0 = np.full(T, NEG, np.float64)
    v0[START] = 0.0
    for s in range(L):
        v0 = _logsumexp(trans[None, :, :] + v0[None, None, :], axis=2)[0] \
            + feats[0, s]
    ident = np.full((NLIVE, NLIVE), -np.inf)
    np.fill_diagonal(ident, 0.0)
    logA[0, 0] = ident

    # tree-combine the KPG chunk mats of each sentence -> one mat per sentence
    mats = logA.reshape(B * KPG, NLIVE, NLIVE)
    n = B * KPG
    while n > B:
        A2 = mats[0::2]        # earlier chunk
        B2 = mats[1::2]        # later chunk
        x = B2[:, :, :, None] + A2[:, None, :, :]
        mats = _logsumexp(x, axis=2)
        n //= 2

    # sequential carry across sentences
    last = np.zeros((B, T), np.float64)
    v = v0.copy()
    for bi in range(B):
        vl = _logsumexp(mats[bi] + v[None, :NLIVE], axis=1)
        v = np.concatenate([vl, [-np.inf, -np.inf]])
        last[bi] = v
    forward_score = _logsumexp(last + trans[STOP][None, :], axis=1)
    err.__exit__(None, None, None)

    tags = np.asarray(tags)
    tags_ext = np.concatenate(
        [np.full((B, 1), START, dtype=tags.dtype), tags], axis=1)
    prev, nxt = tags_ext[:, :-1], tags_ext[:, 1:]
    trans_sc = trans[nxt, prev].sum(axis=1)
    emit_sc = np.take_along_axis(
        feats, nxt[..., None].astype(np.int64), axis=2)[..., 0].sum(axis=1)
    gold = trans_sc + emit_sc + trans[STOP, tags_ext[:, -1]]
    gold_cum = np.cumsum(gold)
    out = np.sum(forward_score - gold_cum)
    return np.array([out], dtype=np.float32)


def kernel(hidden, W, b, transitions, tags, _trace=False, _tmpdir=None):
    hidden = np.asarray(hidden, dtype=np.float32)
    W = np.asarray(W, dtype=np.float32)
    b = np.asarray(b, dtype=np.float32)
    transitions = np.asarray(transitions, dtype=np.float32)
    res = _run_device(hidden, W, b, transitions, trace=_trace, tmpdir=_tmpdir)
    out = _host_combine(res.results, transitions, tags)
    if _trace:
        return out, res
    return out


# revision 65
# speedup vs baseline: 1.0095x; 1.0095x over previous
"""BERT-CRF loss kernel for Trainium2 (8 NeuronCores, data-parallel over positions).

Math: loss = sum_b(forward_b - cumsum(gold)_b) for a CRF whose forward scan runs
over the flattened B*S steps (batch carryover).  The log-semiring scan is
reassociated into per-chunk (L=2 positions) transfer matrices computed on
device in scaled probability space:

  feats[pos,t] = hidden @ W.T + b     (PE, fp8, pos-major)
  m[pos]       = max over live tags   (DVE reduce from PSUM)
  fsub         = feats - m            (DVE sub, bf16, shipped for gold score)
  EF           = exp(fsub)            (ACT, bf16, pair-packed; PE transposes
                                       to tag-major PSUM)
  chunk E:  A = diag(EF_1) Elive diag(EF_0) Elive
            At0 <- e40jm * EF_0 ; ps <- Eblk.T @ At0 ; A <- ps * EF_1
            (one DVE mul + one PE matmul + one DVE mul per chain; EF read
             directly from bf16 PSUM at DVE 2x)

Positions are column-permuted on the host so that each chain step reads a
contiguous 32-column EF slice, and the 8 sentences (groups) are pair-packed
at partition offsets {0,16} inside 32-aligned slots (PE transposes write
[32,128] blocks at legal partition starts).

Every engine instruction and DMA may carry at most ONE semaphore wait on this
target, so multi-dep points are preceded by tiny data-dependent "absorber"
instructions that pull foreign sems into the local engine clock.

Host combines the 16384 chunk matrices (f64, vectorized pairwise tree per
sentence + sequential sentence carry) and computes the gold score from the
shipped fsub + m.
"""
import numpy as np
import ml_dtypes
from contextlib import ExitStack

import concourse.bass as bass
import concourse.mybir as mybir
from concourse.tile import TileContext
from concourse.tile_rust import add_dep_helper
from concourse.bass_utils import run_bass_kernel_spmd

B, S, H, T = 64, 512, 768, 12
START, STOP, NEG = 10, 11, -10000.0
L = 2                      # chunk length (positions per transfer matrix)
NCORES = 8
P_CORE = B * S // NCORES   # 4096 positions per core
G = 8                      # sentences (groups) per core
KPG = S // L               # 256 chunks per group
NE = 8                     # scan chains per core
CPE = KPG // NE            # 32 chunks per group per chain
NLIVE = 10

FP8NP = ml_dtypes.float8_e4m3fn
BF16 = ml_dtypes.bfloat16

F32 = mybir.dt.float32
BF = mybir.dt.bfloat16
FP8 = mybir.dt.float8e4

# device column <-> original position permutation (per core)
# position q = g*512 + k*2 + s ; chain E = k//32 ; c = k%32
# col = (E//2)*1024 + g*128 + (E%2)*64 + s*32 + c
_cols = np.arange(P_CORE)
_P = _cols // 1024                   # quarter (chain pair)
_g = (_cols % 1024) // 128
_in = _cols % 128
_E = _P * 2 + _in // 64
_s = (_in % 64) // 32
_c = _in % 32
PERM = (_g * S + (_E * CPE + _c) * L + _s)   # PERM[col] = original position


def _build_nc():
    nc = bass.Bass()
    hidT = nc.declare_dram_parameter("hidT", [H, P_CORE], FP8, isOutput=False)
    cf8 = nc.declare_dram_parameter("cf8", [128, 212], FP8, isOutput=False)
    cb16 = nc.declare_dram_parameter("cb16", [128, 576], BF, isOutput=False)
    fm_out = nc.declare_dram_parameter("fm_out", [128, 416], BF, isOutput=True)
    a_out = nc.declare_dram_parameter("a_out", [128, NE * 320], BF, isOutput=True)

    with ExitStack() as ctx:
        tc = ctx.enter_context(TileContext(nc))
        const_pool = ctx.enter_context(tc.tile_pool(name="const", bufs=1))
        hid_pool = ctx.enter_context(tc.tile_pool(name="hid", bufs=12))
        sb_pool = ctx.enter_context(tc.tile_pool(name="sb", bufs=1))
        at_pool = ctx.enter_context(tc.tile_pool(name="at", bufs=4))
        fp_pool = ctx.enter_context(tc.tile_pool(name="fps", bufs=1, space="PSUM"))
        eft_pool = ctx.enter_context(tc.tile_pool(name="eftp", bufs=1, space="PSUM"))
        ps_pool = ctx.enter_context(tc.tile_pool(name="pss", bufs=3, space="PSUM"))
        warm_pool = ctx.enter_context(tc.tile_pool(name="warm", bufs=1, space="PSUM"))

        # ---- persistent SBUF tiles ----
        cf8_sb = const_pool.tile([128, 212], FP8)
        cb16_sb = const_pool.tile([128, 576], BF)
        efpos = sb_pool.tile([128, 512], BF)       # pos-major EF, pair-packed
        fraws = [sb_pool.tile([128, 96], BF, name=f"fraw{q}")
                 for q in range(4)]                # feats psum copied to SBUF
        fm_sb = sb_pool.tile([128, 416], BF)       # m (cols 0:32) | fsub (32:416)
        a_sb = sb_pool.tile([128, NE * 320], BF)   # final chunk matrices
        efsbs = [sb_pool.tile([128, 128], BF, name=f"efsb{q}")
                 for q in range(4)]                # tag-major EF (SBUF copy)
        scrqs = [sb_pool.tile([1, 2], BF, name=f"scrq{q}")
                 for q in range(4)]                # per-quarter absorber scratch
        scrps = [sb_pool.tile([1, 2], BF, name=f"scrp{q}")
                 for q in range(4)]                # per-quarter ACT scratch
        scrbs = [sb_pool.tile([1, 2], BF, name=f"scrb{q}")
                 for q in range(4)]                # per-quarter Pool-sub scratch

        eft_ps = eft_pool.tile([128, 512], BF)     # transposed EF psum

        all_dmas = []
        out_dmas = []
        last_insts = {}
        hid = {}

        def hdma(eng, hs, h):
            t = hid_pool.tile([128, 2048], FP8, name=f"hid_{hs}_{h}", tag="hid")
            di = eng.dma_start(
                out=t[:, :],
                in_=hidT[hs * 128:(hs + 1) * 128, h * 2048:(h + 1) * 2048])
            all_dmas.append(di)
            hid[(hs, h)] = t

        # ---- pad init on DVE: zero the 4-wide pad cols of efpos ----
        memset_i = nc.vector.memset(
            bass.AP(efpos.tensor, efpos[:, 12:16].offset,
                    [efpos[:, :].ap[0], [16, 32], [1, 4]]),
            0.0,
        )

        # ---- consts on ACT; hid on Pool/SP ----
        all_dmas.append(nc.scalar.dma_start(out=cf8_sb[:, :], in_=cf8[:, :]))
        all_dmas.append(nc.scalar.dma_start(out=cb16_sb[:, :], in_=cb16[:, :]))
        hdma(nc.gpsimd, 0, 0)
        hdma(nc.sync, 3, 0)
        hdma(nc.gpsimd, 1, 0)
        hdma(nc.sync, 4, 0)
        hdma(nc.gpsimd, 2, 0)
        hdma(nc.sync, 5, 0)

        ident = cb16_sb[:, 0:128]
        eblk = cb16_sb[:, 128:256]
        e40jm = cb16_sb[:, 256:576]

        # ---- warmups: ramp PE, load ACT exp table, absorb const/memset sems
        wp = warm_pool.tile([128, 384], F32)
        scr = const_pool.tile([1, 16], BF)
        nc.scalar.activation(scr[0:1, 0:8], cf8_sb[0:1, 0:8],
                             mybir.ActivationFunctionType.Exp)
        for _ in range(3):
            nc.tensor.matmul(wp[:, 0:212], lhsT=cf8_sb[:, 0:128],
                             rhs=cf8_sb[:, 0:212], start=True, stop=True)
        wm = nc.tensor.matmul(wp[:, 0:128], lhsT=cb16_sb[:, 0:128],
                              rhs=cb16_sb[:, 0:128], start=True, stop=True)
        add_dep_helper(wm.ins, memset_i.ins, True, "absorb memset sem into PE")
        scrp = const_pool.tile([1, 16], BF)
        # absorb the cb16 DMA-queue sem into the DVE clock (for e40jm/s0 muls)
        nc.vector.tensor_copy(scrp[0:1, 8:10], cb16_sb[0:1, 256:258])

        fps = {}

        def feats_mms(Q):
            h = Q // 2
            fp_ps = fp_pool.tile([128, 96], F32, name=f"fp{Q}", tag="fp")
            fps[Q] = fp_ps
            if Q >= 1:
                # absorber pair for the recycled feats-psum bank
                ab0 = nc.tensor.matmul(wp[0:1, 0:1], lhsT=cf8_sb[0:1, 0:1],
                                       rhs=fraws[Q - 1][0:1, 0:1],
                                       start=True, stop=True)
                ab1 = nc.tensor.matmul(fp_ps[0:1, 0:1], lhsT=cf8_sb[0:1, 0:1],
                                       rhs=cf8_sb[0:1, 0:1], start=True,
                                       stop=True, skip_group_check=True)
                add_dep_helper(ab1.ins, ab0.ins, False, "absorber ordering")
            for g in range(G):
                blk = fp_ps[:, g * 12:(g + 1) * 12]
                for hs in range(6):
                    nc.tensor.matmul(
                        blk,
                        lhsT=hid[(hs, h)][:, (Q % 2) * 1024 + g * 128:
                                          (Q % 2) * 1024 + (g + 1) * 128],
                        rhs=cf8_sb[:, hs * 12:(hs + 1) * 12],
                        start=(hs == 0), stop=False,
                        skip_group_check=True,
                    )
                nc.tensor.matmul(
                    blk, lhsT=cf8_sb[0:1, 72:200], rhs=cf8_sb[0:1, 200:212],
                    start=False, stop=True, skip_group_check=True,
                )

        def feats_tail(Q):
            fp_ps = fps[Q]
            fraw = fraws[Q]
            # copy feats psum -> SBUF bf16 (ACT)
            cp_i = nc.scalar.activation(
                fraw[:, :], fp_ps[:, :], mybir.ActivationFunctionType.Copy)
            last_insts['actcp'] = cp_i
            # m = max over live tags (DVE, bf16 2x)
            fq3 = fraw[:, :].rearrange("p (b j) -> p b j", j=12)
            nc.vector.reduce_max(
                out=fm_sb[:, Q * 8:(Q + 1) * 8],
                in_=bass.AP(fq3.tensor, fq3.offset,
                            [fq3.ap[0], fq3.ap[1], [1, NLIVE]]),
                axis=mybir.AxisListType.X,
            )
            # fsub = feats - m (DVE, all SBUF bf16)
            msl = fm_sb[:, Q * 8:(Q + 1) * 8]
            m_b = bass.AP(msl.tensor, msl.offset,
                          [msl.ap[0], msl.ap[1], [0, 12]])
            if Q >= 2:
                # B-half subs run on Pool (idle after its DMAs) so they do
                # not contend with the chain muls on DVE.  Absorber pulls the
                # DVE (reduce) sem into the Pool clock first.
                nc.gpsimd.tensor_copy(scrbs[Q][0:1, 0:2],
                                      fm_sb[0:1, Q * 8:Q * 8 + 2])
                last_insts['pool'] = nc.gpsimd.tensor_sub(
                    fm_sb[:, 32 + Q * 96:32 + (Q + 1) * 96]
                    .rearrange("p (b j) -> p b j", j=12),
                    fq3, m_b,
                )
            else:
                last_insts['dve'] = nc.vector.tensor_sub(
                    fm_sb[:, 32 + Q * 96:32 + (Q + 1) * 96]
                    .rearrange("p (b j) -> p b j", j=12),
                    fq3, m_b,
                )
            # EF = exp(fsub) into pair-packed layout (ACT)
            eo = efpos[:, Q * 128:(Q + 1) * 128]
            eo3 = bass.AP(eo.tensor, eo.offset, [eo.ap[0], [16, 8], [1, 12]])
            last_insts['act'] = nc.scalar.activation(
                eo3,
                fm_sb[:, 32 + Q * 96:32 + (Q + 1) * 96]
                .rearrange("p (b j) -> p b j", j=12),
                mybir.ActivationFunctionType.Exp,
            )

        def transposes(Q):
            for v in range(4):
                nc.tensor.transpose(
                    eft_ps[32 * v:32 * v + 32, Q * 128:(Q + 1) * 128],
                    efpos[:, Q * 128 + v * 32:Q * 128 + v * 32 + 32],
                    ident,
                    tile_position=(0, 32 * v),
                )
            # s=1 EF slices must live in SBUF: the s1 mul already reads the
            # matmul output from PSUM and HW allows only one PSUM input.
            # (tiny absorber first: pulls the PE transpose sem into the DVE
            # clock so the copy itself needs at most one wait)
            nc.vector.tensor_copy(scrqs[Q][0:1, 0:2],
                                  eft_ps[0:1, Q * 128 + 32:Q * 128 + 34])
            nc.vector.tensor_copy(ef1s[Q][:, :],
                                  eft_ps[:, Q * 128 + 32:Q * 128 + 128])

        def ef_ap(E, s):
            if s == 0:
                col = (E // 2) * 128 + (E % 2) * 64
                base = eft_ps[:, col:col + 32]
            else:
                col = (E % 2) * 64
                base = ef1s[E // 2][:, col:col + 32]
            return bass.AP(base.tensor, base.offset,
                           [base.ap[0], [0, NLIVE], base.ap[1]])

        ats = {}

        def chain_s0_pair(Q):
            # one DVE op computes both chains' s0 for the quarter:
            # at2[p, j*64 + e*32 + c] = e40jm[p, j*32+c] * EF[p, e*64+c]
            at2 = at_pool.tile([128, 640], BF, name=f"at_{Q}", tag="at")
            ef = efsbs[Q]
            nc.vector.tensor_mul(
                bass.AP(at2.tensor, at2[:, :].offset,
                        [at2[:, :].ap[0], [32, NLIVE], [320, 2], [1, 32]]),
                bass.AP(cb16_sb.tensor, e40jm.offset,
                        [e40jm.ap[0], [32, NLIVE], [0, 2], [1, 32]]),
                bass.AP(ef.tensor, ef[:, :].offset,
                        [ef[:, :].ap[0], [0, NLIVE], [64, 2], [1, 32]]),
            )
            ats[2 * Q] = at2
            ats[2 * Q + 1] = at2

        ps6_sb = sb_pool.tile([128, 320], BF)      # E6 ps staged via ACT
        pmuls = {}

        def chain_s1(E):
            at2 = ats[E]
            at = at2[:, (E % 2) * 320:(E % 2) * 320 + 320]
            # absorber 1: pull the DVE (At ready) sem into the PE clock
            ab1 = nc.tensor.matmul(wp[0:1, 0:1], lhsT=cb16_sb[0:1, 0:1],
                                   rhs=at[0:1, 0:1], start=True, stop=True)
            ps = ps_pool.tile([128, 320], F32)
            # absorber 2: dummy first-writer carries the PSUM bank hazard
            ab2 = nc.tensor.matmul(ps[0:1, 0:1], lhsT=cb16_sb[0:1, 0:1],
                                   rhs=cb16_sb[0:1, 0:1], start=True,
                                   stop=True, skip_group_check=True)
            add_dep_helper(ab2.ins, ab1.ins, False, "absorber ordering")
            last_insts['pe'] = nc.tensor.matmul(
                ps[:, :], lhsT=eblk, rhs=at,
                start=True, stop=True, skip_group_check=True)
            if E == 6:
                # late-window offload: ACT stages ps to SBUF, Pool does the
                # mul - takes one 458ns mul off the saturated DVE tail
                last_insts['actps'] = nc.scalar.activation(
                    ps6_sb[:, :], ps[:, :], mybir.ActivationFunctionType.Copy)
                pmuls['e6'] = nc.gpsimd.tensor_mul(
                    a_sb[:, E * 320:(E + 1) * 320]
                    .rearrange("p (j c) -> p j c", c=32),
                    ps6_sb[:, :].rearrange("p (j c) -> p j c", c=32),
                    ef_ap(E, 1),
                )
                last_insts['pool'] = pmuls['e6']
            else:
                last_insts['dve'] = nc.vector.tensor_mul(
                    a_sb[:, E * 320:(E + 1) * 320]
                    .rearrange("p (j c) -> p j c", c=32),
                    ps[:, :].rearrange("p (j c) -> p j c", c=32),
                    ef_ap(E, 1),
                )

        def aout(lo, hi):
            # Pool absorber pulls the (DVE) producer sem in, then the SWDGE
            # DMA needs only its queue wait
            if lo == 1920 and 'e6' in pmuls:
                # also pre-absorb the Pool-self sem of the offloaded E6 mul
                # (SWDGE transfers do not respect engine order)
                abp = nc.gpsimd.tensor_copy(scrp[0:1, 12:14],
                                            scrbs[2][0:1, 0:2])
                add_dep_helper(abp.ins, pmuls['e6'].ins, True,
                               "absorb E6 pool mul for aout")
            last_insts['poolab'] = nc.gpsimd.tensor_copy(
                scrp[0:1, (lo // 640) * 2:(lo // 640) * 2 + 2],
                a_sb[0:1, hi - 2:hi])
            out_dmas.append(nc.gpsimd.dma_start(out=a_out[:, lo:hi],
                                                in_=a_sb[:, lo:hi]))

        # ================= emission order =================
        feats_mms(0)
        feats_mms(1)
        hdma(nc.gpsimd, 0, 1)
        hdma(nc.sync, 3, 1)
        feats_tail(0)
        hdma(nc.gpsimd, 1, 1)
        hdma(nc.sync, 4, 1)
        feats_tail(1)
        hdma(nc.gpsimd, 2, 1)
        hdma(nc.sync, 5, 1)
        transposes(0)
        transposes(1)
        chain_s0_pair(0)
        chain_s0_pair(1)
        chain_s1(0)
        feats_mms(2)
        feats_tail(2)
        chain_s1(1)
        feats_mms(3)
        feats_tail(3)
        chain_s1(2)
        chain_s1(3)
        aout(0, 1280)
        # fm_out after the last sub (Q3)
        nc.gpsimd.tensor_copy(scrp[0:1, 14:16], fm_sb[0:1, 414:416])
        out_dmas.append(nc.gpsimd.dma_start(out=fm_out[:, :], in_=fm_sb[:, :]))
        transposes(2)
        transposes(3)
        chain_s0_pair(2)
        chain_s0_pair(3)
        chain_s1(4)
        chain_s1(5)
        aout(1280, 1920)
        chain_s1(6)
        chain_s1(7)
        aout(1920, 2560)
        # Pre-absorb every proc's clock into SP one dep at a time, so the
        # Tile tail drain does not need a multi-sem wait.
        for dep in all_dmas + list(last_insts.values()) + out_dmas:
            nop = nc.sync.nop()
            add_dep_helper(nop.ins, dep.ins, True, "drain preclear")
    return nc


_NC_CACHE = None


def _get_nc():
    global _NC_CACHE
    if _NC_CACHE is None:
        _NC_CACHE = _build_nc()
    return _NC_CACHE


def _build_consts(W, b, transitions):
    E = np.exp(transitions.astype(np.float64))
    E[START, :] = 0.0
    E[STOP, :] = 0.0
    E[:, STOP] = 0.0
    E = E.astype(np.float32)

    cf8 = np.zeros((128, 212), np.float32)
    cf8[:, 0:72] = W.T.reshape(6, 128, T).transpose(1, 0, 2).reshape(128, 72)
    cf8[0, 72:200] = 1.0
    cf8[0, 200:212] = b
    cf8 = cf8.astype(FP8NP)

    cb16 = np.zeros((128, 576), np.float32)
    cb16[:, 0:128] = np.eye(128)
    # eblk: eblk[32v+off+j, 32v+off+i] = E[i, j]  (live 10x10)
    for v in range(4):
        for off in (0, 16):
            o = 32 * v + off
            cb16[o:o + NLIVE, 128 + o:128 + o + NLIVE] = E[:NLIVE, :NLIVE].T
    # e40jm: e40jm[32v+off+i, j*32+c] = E[i, j]
    blk = np.zeros((32, 320), np.float32)
    for off in (0, 16):
        for i in range(NLIVE):
            for j in range(NLIVE):
                blk[off + i, j * 32:(j + 1) * 32] = E[i, j]
    for v in range(4):
        cb16[32 * v:32 * v + 32, 256:576] = blk
    cb16 = cb16.astype(BF16)
    return cf8, cb16


def _run_device(hidden, W, b, transitions, trace=False, tmpdir=None):
    cf8, cb16 = _build_consts(W, b, transitions)
    flat = hidden.reshape(B * S, H)
    in_maps = []
    for core in range(NCORES):
        blk = flat[core * P_CORE:(core + 1) * P_CORE]        # [4096, 768]
        hT = np.ascontiguousarray(blk[PERM].T).astype(FP8NP)  # [768, 4096]
        in_maps.append({"hidT": hT, "cf8": cf8, "cb16": cb16})
    return run_bass_kernel_spmd(
        _get_nc(), in_maps, list(range(NCORES)), trace=trace, tmpdir=tmpdir)


def _logsumexp(x, axis):
    mx = np.max(x, axis=axis)
    mx_safe = np.where(np.isfinite(mx), mx, 0.0)
    out = mx + np.log(np.sum(np.exp(x - np.expand_dims(mx_safe, axis)), axis=axis))
    return np.where(np.isfinite(mx), out, -np.inf)


def _host_combine(results, transitions, tags):
    trans = transitions.astype(np.float64)
    err = np.errstate(invalid="ignore", divide="ignore", over="ignore")
    err.__enter__()

    # unpack fm_out: m and fsub in device (permuted) order -> original order
    feats = np.zeros((NCORES, P_CORE, T), np.float64)
    m_all = np.zeros((NCORES, P_CORE), np.float64)
    tt = np.arange(32)
    pp = np.arange(128)
    colidx = (tt[None, :] // 8) * 1024 + (tt[None, :] % 8) * 128 + pp[:, None]
    q_of = PERM[colidx]                # [p, t] original position
    for core, r in enumerate(results):
        fm = np.asarray(r["fm_out"]).astype(np.float64)   # [128, 416]
        m_dev = fm[:, 0:32]            # [p, t]
        fs_dev = fm[:, 32:416].reshape(128, 32, 12)   # [p, t, j]
        m_all[core, q_of] = m_dev
        feats[core, q_of, :] = fs_dev + m_dev[:, :, None]

    feats = feats.reshape(B, S, T)
    m_all = m_all.reshape(B, S)

    # unpack chunk matrices: A[b, k][i, j] (live 10x10), log + scale
    logA = np.zeros((B, KPG, NLIVE, NLIVE), np.float64)
    for core, r in enumerate(results):
        a = np.asarray(r["a_out"]).astype(np.float64)     # [128, 2560]
        a4 = a.reshape(128, NE, NLIVE, CPE)                # [p, E, j, c]
        for g in range(G):
            rows = 32 * (g // 2) + 16 * (g % 2)
            blkm = a4[rows:rows + NLIVE]                   # [i, E, j, c]
            logA[core * G + g] = np.log(blkm).transpose(1, 3, 0, 2).reshape(
                KPG, NLIVE, NLIVE)
    scale = m_all.reshape(B, KPG, L).sum(axis=2)           # [B, KPG]
    logA = logA + scale[:, :, None, None]

    # first global chunk: explicit recurrence from init (full 12-state)
    v0 = np.full(T, NEG, np.float64)
    v0[START] = 0.0
    for s in range(L):
        v0 = _logsumexp(trans[None, :, :] + v0[None, None, :], axis=2)[0] \
            + feats[0, s]
    ident = np.full((NLIVE, NLIVE), -np.inf)
    np.fill_diagonal(ident, 0.0)
    logA[0, 0] = ident

    # tree-combine the KPG chunk mats of each sentence -> one mat per sentence
    mats = logA.reshape(B * KPG, NLIVE, NLIVE)
    n = B * KPG
    while n > B:
        A2 = mats[0::2]        # earlier chunk
        B2 = mats[1::2]        # later chunk
        x = B2[:, :, :, None] + A2[:, None, :, :]
        mats = _logsumexp(x, axis=2)
        n //= 2

    # sequential carry across sentences
    last = np.zeros((B, T), np.float64)
    v = v0.copy()
    for bi in range(B):
        vl = _logsumexp(mats[bi] + v[None, :NLIVE], axis=1)
        v = np.concatenate([vl, [-np.inf, -np.inf]])
        last[bi] = v
    forward_score = _logsumexp(last + trans[STOP][None, :], axis=1)
    err.__exit__(None, None, None)

    tags = np.asarray(tags)
    tags_ext = np.concatenate(
        [np.full((B, 1), START, dtype=tags.dtype), tags], axis=1)
    prev, nxt = tags_ext[:, :-1], tags_ext[:, 1:]
    trans_sc = trans[nxt, prev].sum(axis=1)
    emit_sc = np.take_along_axis(
        feats, nxt[..., None].astype(np.int64), axis=2)[..., 0].sum(axis=1)
    gold = trans_sc + emit_sc + trans[STOP, tags_ext[:, -1]]
    gold_cum = np.cumsum(gold)
    out = np.sum(forward_score - gold_cum)
    return np.array([out], dtype=np.float32)


def kernel(hidden, W, b, transitions, tags, _trace=False, _tmpdir=None):
    hidden = np.asarray(hidden, dtype=np.float32)
    W = np.asarray(W, dtype=np.float32)
    b = np.asarray(b, dtype=np.float32)
    transitions = np.asarray(transitions, dtype=np.float32)
    res = _run_device(hidden, W, b, transitions, trace=_trace, tmpdir=_tmpdir)
    out = _host_combine(res.results, transitions, tags)
    if _trace:
        return out, res
    return out


# revision 66
# speedup vs baseline: 1.0383x; 1.0285x over previous
"""BERT-CRF loss kernel for Trainium2 (8 NeuronCores, data-parallel over positions).

Math: loss = sum_b(forward_b - cumsum(gold)_b) for a CRF whose forward scan runs
over the flattened B*S steps (batch carryover).  The log-semiring scan is
reassociated into per-chunk (L=2 positions) transfer matrices computed on
device in scaled probability space:

  feats[pos,t] = hidden @ W.T + b     (PE, fp8, pos-major)
  m[pos]       = max over live tags   (DVE reduce from PSUM)
  fsub         = feats - m            (DVE sub, bf16, shipped for gold score)
  EF           = exp(fsub)            (ACT, bf16, pair-packed; PE transposes
                                       to tag-major PSUM)
  chunk E:  A = diag(EF_1) Elive diag(EF_0) Elive
            At0 <- e40jm * EF_0 ; ps <- Eblk.T @ At0 ; A <- ps * EF_1
            (one DVE mul + one PE matmul + one DVE mul per chain; EF read
             directly from bf16 PSUM at DVE 2x)

Positions are column-permuted on the host so that each chain step reads a
contiguous 32-column EF slice, and the 8 sentences (groups) are pair-packed
at partition offsets {0,16} inside 32-aligned slots (PE transposes write
[32,128] blocks at legal partition starts).

Every engine instruction and DMA may carry at most ONE semaphore wait on this
target, so multi-dep points are preceded by tiny data-dependent "absorber"
instructions that pull foreign sems into the local engine clock.

Host combines the 16384 chunk matrices (f64, vectorized pairwise tree per
sentence + sequential sentence carry) and computes the gold score from the
shipped fsub + m.
"""
import numpy as np
import ml_dtypes
from contextlib import ExitStack

import concourse.bass as bass
import concourse.mybir as mybir
from concourse.tile import TileContext
from concourse.tile_rust import add_dep_helper
from concourse.bass_utils import run_bass_kernel_spmd

B, S, H, T = 64, 512, 768, 12
START, STOP, NEG = 10, 11, -10000.0
L = 2                      # chunk length (positions per transfer matrix)
NCORES = 8
P_CORE = B * S // NCORES   # 4096 positions per core
G = 8                      # sentences (groups) per core
KPG = S // L               # 256 chunks per group
NE = 8                     # scan chains per core
CPE = KPG // NE            # 32 chunks per group per chain
NLIVE = 10

FP8NP = ml_dtypes.float8_e4m3fn
BF16 = ml_dtypes.bfloat16

F32 = mybir.dt.float32
BF = mybir.dt.bfloat16
FP8 = mybir.dt.float8e4

# device column <-> original position permutation (per core)
# position q = g*512 + k*2 + s ; chain E = k//32 ; c = k%32
# col = (E//2)*1024 + g*128 + (E%2)*64 + s*32 + c
_cols = np.arange(P_CORE)
_P = _cols // 1024                   # quarter (chain pair)
_g = (_cols % 1024) // 128
_in = _cols % 128
_E = _P * 2 + _in // 64
_s = (_in % 64) // 32
_c = _in % 32
PERM = (_g * S + (_E * CPE + _c) * L + _s)   # PERM[col] = original position


def _build_nc():
    nc = bass.Bass()
    hidT = nc.declare_dram_parameter("hidT", [H, P_CORE], FP8, isOutput=False)
    cf8 = nc.declare_dram_parameter("cf8", [128, 212], FP8, isOutput=False)
    cb16 = nc.declare_dram_parameter("cb16", [128, 576], BF, isOutput=False)
    fm_out = nc.declare_dram_parameter("fm_out", [128, 416], BF, isOutput=True)
    a_out = nc.declare_dram_parameter("a_out", [128, NE * 320], BF, isOutput=True)

    with ExitStack() as ctx:
        tc = ctx.enter_context(TileContext(nc))
        const_pool = ctx.enter_context(tc.tile_pool(name="const", bufs=1))
        hid_pool = ctx.enter_context(tc.tile_pool(name="hid", bufs=12))
        sb_pool = ctx.enter_context(tc.tile_pool(name="sb", bufs=1))
        at_pool = ctx.enter_context(tc.tile_pool(name="at", bufs=4))
        fp_pool = ctx.enter_context(tc.tile_pool(name="fps", bufs=1, space="PSUM"))
        eft_pool = ctx.enter_context(tc.tile_pool(name="eftp", bufs=1, space="PSUM"))
        ps_pool = ctx.enter_context(tc.tile_pool(name="pss", bufs=3, space="PSUM"))
        warm_pool = ctx.enter_context(tc.tile_pool(name="warm", bufs=1, space="PSUM"))

        # ---- persistent SBUF tiles ----
        cf8_sb = const_pool.tile([128, 212], FP8)
        cb16_sb = const_pool.tile([128, 576], BF)
        efpos = sb_pool.tile([128, 512], BF)       # pos-major EF, pair-packed
        fraws = [sb_pool.tile([128, 96], BF, name=f"fraw{q}")
                 for q in range(4)]                # feats psum copied to SBUF
        fm_sb = sb_pool.tile([128, 416], BF)       # m (cols 0:32) | fsub (32:416)
        a_sb = sb_pool.tile([128, NE * 320], BF)   # final chunk matrices
        efsbs = [sb_pool.tile([128, 128], BF, name=f"efsb{q}")
                 for q in range(4)]                # tag-major EF (SBUF copy)
        scrqs = [sb_pool.tile([1, 2], BF, name=f"scrq{q}")
                 for q in range(4)]                # per-quarter absorber scratch
        scrps = [sb_pool.tile([1, 2], BF, name=f"scrp{q}")
                 for q in range(4)]                # per-quarter ACT scratch
        scrbs = [sb_pool.tile([1, 2], BF, name=f"scrb{q}")
                 for q in range(4)]                # per-quarter Pool-sub scratch

        eft_ps = eft_pool.tile([128, 512], BF)     # transposed EF psum

        all_dmas = []
        out_dmas = []
        last_insts = {}
        hid = {}

        def hdma(eng, hs, h):
            t = hid_pool.tile([128, 2048], FP8, name=f"hid_{hs}_{h}", tag="hid")
            di = eng.dma_start(
                out=t[:, :],
                in_=hidT[hs * 128:(hs + 1) * 128, h * 2048:(h + 1) * 2048])
            all_dmas.append(di)
            hid[(hs, h)] = t

        # ---- pad init on DVE: zero the 4-wide pad cols of efpos ----
        memset_i = nc.vector.memset(
            bass.AP(efpos.tensor, efpos[:, 12:16].offset,
                    [efpos[:, :].ap[0], [16, 32], [1, 4]]),
            0.0,
        )

        # ---- consts on ACT; hid on Pool/SP ----
        all_dmas.append(nc.scalar.dma_start(out=cf8_sb[:, :], in_=cf8[:, :]))
        all_dmas.append(nc.scalar.dma_start(out=cb16_sb[:, :], in_=cb16[:, :]))
        hdma(nc.gpsimd, 0, 0)
        hdma(nc.sync, 3, 0)
        hdma(nc.gpsimd, 1, 0)
        hdma(nc.sync, 4, 0)
        hdma(nc.gpsimd, 2, 0)
        hdma(nc.sync, 5, 0)

        ident = cb16_sb[:, 0:128]
        eblk = cb16_sb[:, 128:256]
        e40jm = cb16_sb[:, 256:576]

        # ---- warmups: ramp PE, load ACT exp table, absorb const/memset sems
        wp = warm_pool.tile([128, 384], F32)
        scr = const_pool.tile([1, 16], BF)
        nc.scalar.activation(scr[0:1, 0:8], cf8_sb[0:1, 0:8],
                             mybir.ActivationFunctionType.Exp)
        for _ in range(3):
            nc.tensor.matmul(wp[:, 0:212], lhsT=cf8_sb[:, 0:128],
                             rhs=cf8_sb[:, 0:212], start=True, stop=True)
        wm = nc.tensor.matmul(wp[:, 0:128], lhsT=cb16_sb[:, 0:128],
                              rhs=cb16_sb[:, 0:128], start=True, stop=True)
        add_dep_helper(wm.ins, memset_i.ins, True, "absorb memset sem into PE")
        scrp = const_pool.tile([1, 16], BF)
        # absorb the cb16 DMA-queue sem into the DVE clock (for e40jm/s0 muls)
        nc.vector.tensor_copy(scrp[0:1, 8:10], cb16_sb[0:1, 256:258])

        fps = {}

        def feats_mms(Q):
            h = Q // 2
            fp_ps = fp_pool.tile([128, 96], F32, name=f"fp{Q}", tag="fp")
            fps[Q] = fp_ps
            if Q >= 1:
                # absorber pair for the recycled feats-psum bank
                ab0 = nc.tensor.matmul(wp[0:1, 0:1], lhsT=cf8_sb[0:1, 0:1],
                                       rhs=fraws[Q - 1][0:1, 0:1],
                                       start=True, stop=True)
                ab1 = nc.tensor.matmul(fp_ps[0:1, 0:1], lhsT=cf8_sb[0:1, 0:1],
                                       rhs=cf8_sb[0:1, 0:1], start=True,
                                       stop=True, skip_group_check=True)
                add_dep_helper(ab1.ins, ab0.ins, False, "absorber ordering")
            for g in range(G):
                blk = fp_ps[:, g * 12:(g + 1) * 12]
                for hs in range(6):
                    nc.tensor.matmul(
                        blk,
                        lhsT=hid[(hs, h)][:, (Q % 2) * 1024 + g * 128:
                                          (Q % 2) * 1024 + (g + 1) * 128],
                        rhs=cf8_sb[:, hs * 12:(hs + 1) * 12],
                        start=(hs == 0), stop=False,
                        skip_group_check=True,
                    )
                nc.tensor.matmul(
                    blk, lhsT=cf8_sb[0:1, 72:200], rhs=cf8_sb[0:1, 200:212],
                    start=False, stop=True, skip_group_check=True,
                )

        def feats_tail(Q):
            fp_ps = fps[Q]
            fraw = fraws[Q]
            # copy feats psum -> SBUF bf16 (ACT)
            cp_i = nc.scalar.activation(
                fraw[:, :], fp_ps[:, :], mybir.ActivationFunctionType.Copy)
            last_insts['actcp'] = cp_i
            # m = max over live tags (DVE, bf16 2x)
            fq3 = fraw[:, :].rearrange("p (b j) -> p b j", j=12)
            nc.vector.reduce_max(
                out=fm_sb[:, Q * 8:(Q + 1) * 8],
                in_=bass.AP(fq3.tensor, fq3.offset,
                            [fq3.ap[0], fq3.ap[1], [1, NLIVE]]),
                axis=mybir.AxisListType.X,
            )
            # fsub = feats - m (DVE, all SBUF bf16)
            msl = fm_sb[:, Q * 8:(Q + 1) * 8]
            m_b = bass.AP(msl.tensor, msl.offset,
                          [msl.ap[0], msl.ap[1], [0, 12]])
            if Q >= 2:
                # B-half subs run on Pool (idle after its DMAs) so they do
                # not contend with the chain muls on DVE.  Absorber pulls the
                # DVE (reduce) sem into the Pool clock first.
                nc.gpsimd.tensor_copy(scrbs[Q][0:1, 0:2],
                                      fm_sb[0:1, Q * 8:Q * 8 + 2])
                last_insts['pool'] = nc.gpsimd.tensor_sub(
                    fm_sb[:, 32 + Q * 96:32 + (Q + 1) * 96]
                    .rearrange("p (b j) -> p b j", j=12),
                    fq3, m_b,
                )
            else:
                last_insts['dve'] = nc.vector.tensor_sub(
                    fm_sb[:, 32 + Q * 96:32 + (Q + 1) * 96]
                    .rearrange("p (b j) -> p b j", j=12),
                    fq3, m_b,
                )
            # EF = exp(fsub) into pair-packed layout (ACT)
            eo = efpos[:, Q * 128:(Q + 1) * 128]
            eo3 = bass.AP(eo.tensor, eo.offset, [eo.ap[0], [16, 8], [1, 12]])
            last_insts['act'] = nc.scalar.activation(
                eo3,
                fm_sb[:, 32 + Q * 96:32 + (Q + 1) * 96]
                .rearrange("p (b j) -> p b j", j=12),
                mybir.ActivationFunctionType.Exp,
            )

        def transposes(Q):
            for v in range(4):
                nc.tensor.transpose(
                    eft_ps[32 * v:32 * v + 32, Q * 128:(Q + 1) * 128],
                    efpos[:, Q * 128 + v * 32:Q * 128 + v * 32 + 32],
                    ident,
                    tile_position=(0, 32 * v),
                )
            # s=1 EF slices must live in SBUF: the s1 mul already reads the
            # matmul output from PSUM and HW allows only one PSUM input.
            # (tiny absorber first: pulls the PE transpose sem into the DVE
            # clock so the copy itself needs at most one wait)
            nc.vector.tensor_copy(scrqs[Q][0:1, 0:2],
                                  eft_ps[0:1, Q * 128 + 32:Q * 128 + 34])
            nc.vector.tensor_copy(ef1s[Q][:, :],
                                  eft_ps[:, Q * 128 + 32:Q * 128 + 128])

        def ef_ap(E, s):
            if s == 0:
                col = (E // 2) * 128 + (E % 2) * 64
                base = eft_ps[:, col:col + 32]
            else:
                col = (E % 2) * 64
                base = ef1s[E // 2][:, col:col + 32]
            return bass.AP(base.tensor, base.offset,
                           [base.ap[0], [0, NLIVE], base.ap[1]])

        ats = {}

        def chain_s0_pair(Q):
            # one DVE op computes both chains' s0 for the quarter:
            # at2[p, j*64 + e*32 + c] = e40jm[p, j*32+c] * EF[p, e*64+c]
            at2 = at_pool.tile([128, 640], BF, name=f"at_{Q}", tag="at")
            ef = efsbs[Q]
            nc.vector.tensor_mul(
                bass.AP(at2.tensor, at2[:, :].offset,
                        [at2[:, :].ap[0], [32, NLIVE], [320, 2], [1, 32]]),
                bass.AP(cb16_sb.tensor, e40jm.offset,
                        [e40jm.ap[0], [32, NLIVE], [0, 2], [1, 32]]),
                bass.AP(ef.tensor, ef[:, :].offset,
                        [ef[:, :].ap[0], [0, NLIVE], [64, 2], [1, 32]]),
            )
            ats[2 * Q] = at2
            ats[2 * Q + 1] = at2

        ps6_sb = sb_pool.tile([128, 320], BF)      # E6 ps staged via ACT
        pmuls = {}

        def chain_s1(E):
            at2 = ats[E]
            at = at2[:, (E % 2) * 320:(E % 2) * 320 + 320]
            # absorber 1: pull the DVE (At ready) sem into the PE clock
            ab1 = nc.tensor.matmul(wp[0:1, 0:1], lhsT=cb16_sb[0:1, 0:1],
                                   rhs=at[0:1, 0:1], start=True, stop=True)
            ps = ps_pool.tile([128, 320], F32)
            # absorber 2: dummy first-writer carries the PSUM bank hazard
            ab2 = nc.tensor.matmul(ps[0:1, 0:1], lhsT=cb16_sb[0:1, 0:1],
                                   rhs=cb16_sb[0:1, 0:1], start=True,
                                   stop=True, skip_group_check=True)
            add_dep_helper(ab2.ins, ab1.ins, False, "absorber ordering")
            last_insts['pe'] = nc.tensor.matmul(
                ps[:, :], lhsT=eblk, rhs=at,
                start=True, stop=True, skip_group_check=True)
            if E == 6:
                # late-window offload: ACT stages ps to SBUF, Pool does the
                # mul - takes one 458ns mul off the saturated DVE tail
                last_insts['actps'] = nc.scalar.activation(
                    ps6_sb[:, :], ps[:, :], mybir.ActivationFunctionType.Copy)
                pmuls['e6'] = nc.gpsimd.tensor_mul(
                    a_sb[:, E * 320:(E + 1) * 320]
                    .rearrange("p (j c) -> p j c", c=32),
                    ps6_sb[:, :].rearrange("p (j c) -> p j c", c=32),
                    ef_ap(E, 1),
                )
                last_insts['pool'] = pmuls['e6']
            else:
                last_insts['dve'] = nc.vector.tensor_mul(
                    a_sb[:, E * 320:(E + 1) * 320]
                    .rearrange("p (j c) -> p j c", c=32),
                    ps[:, :].rearrange("p (j c) -> p j c", c=32),
                    ef_ap(E, 1),
                )

        def aout(lo, hi):
            # Pool absorber pulls the (DVE) producer sem in, then the SWDGE
            # DMA needs only its queue wait
            if lo == 1920 and 'e6' in pmuls:
                # also pre-absorb the Pool-self sem of the offloaded E6 mul
                # (SWDGE transfers do not respect engine order)
                abp = nc.gpsimd.tensor_copy(scrp[0:1, 12:14],
                                            scrbs[2][0:1, 0:2])
                add_dep_helper(abp.ins, pmuls['e6'].ins, True,
                               "absorb E6 pool mul for aout")
            last_insts['poolab'] = nc.gpsimd.tensor_copy(
                scrp[0:1, (lo // 640) * 2:(lo // 640) * 2 + 2],
                a_sb[0:1, hi - 2:hi])
            out_dmas.append(nc.gpsimd.dma_start(out=a_out[:, lo:hi],
                                                in_=a_sb[:, lo:hi]))

        # ================= emission order =================
        feats_mms(0)
        feats_mms(1)
        hdma(nc.gpsimd, 0, 1)
        hdma(nc.sync, 3, 1)
        feats_tail(0)
        hdma(nc.gpsimd, 1, 1)
        hdma(nc.sync, 4, 1)
        feats_tail(1)
        hdma(nc.gpsimd, 2, 1)
        hdma(nc.sync, 5, 1)
        transposes(0)
        transposes(1)
        chain_s0_pair(0)
        chain_s0_pair(1)
        chain_s1(0)
        feats_mms(2)
        feats_tail(2)
        chain_s1(1)
        feats_mms(3)
        feats_tail(3)
        chain_s1(2)
        chain_s1(3)
        aout(0, 1280)
        # fm_out after the last sub (Q3)
        nc.gpsimd.tensor_copy(scrp[0:1, 14:16], fm_sb[0:1, 414:416])
        out_dmas.append(nc.gpsimd.dma_start(out=fm_out[:, :], in_=fm_sb[:, :]))
        transposes(2)
        transposes(3)
        chain_s0_pair(2)
        chain_s0_pair(3)
        chain_s1(4)
        chain_s1(5)
        # E4/E5 matrices go out via ACT (idle here) so the Pool queue is free
        # for the final E6/E7 DMA; ACT absorber carries the DVE wait
        nc.scalar.activation(scrps[0][0:1, 0:2], a_sb[0:1, 1918:1920],
                             mybir.ActivationFunctionType.Copy)
        out_dmas.append(nc.scalar.dma_start(out=a_out[:, 1280:1920],
                                            in_=a_sb[:, 1280:1920]))
        chain_s1(6)
        chain_s1(7)
        aout(1920, 2560)
        # Pre-absorb every proc's clock into SP one dep at a time, so the
        # Tile tail drain does not need a multi-sem wait.
        for dep in all_dmas + list(last_insts.values()) + out_dmas:
            nop = nc.sync.nop()
            add_dep_helper(nop.ins, dep.ins, True, "drain preclear")
    return nc


_NC_CACHE = None


def _get_nc():
    global _NC_CACHE
    if _NC_CACHE is None:
        _NC_CACHE = _build_nc()
    return _NC_CACHE


def _build_consts(W, b, transitions):
    E = np.exp(transitions.astype(np.float64))
    E[START, :] = 0.0
    E[STOP, :] = 0.0
    E[:, STOP] = 0.0
    E = E.astype(np.float32)

    cf8 = np.zeros((128, 212), np.float32)
    cf8[:, 0:72] = W.T.reshape(6, 128, T).transpose(1, 0, 2).reshape(128, 72)
    cf8[0, 72:200] = 1.0
    cf8[0, 200:212] = b
    cf8 = cf8.astype(FP8NP)

    cb16 = np.zeros((128, 576), np.float32)
    cb16[:, 0:128] = np.eye(128)
    # eblk: eblk[32v+off+j, 32v+off+i] = E[i, j]  (live 10x10)
    for v in range(4):
        for off in (0, 16):
            o = 32 * v + off
            cb16[o:o + NLIVE, 128 + o:128 + o + NLIVE] = E[:NLIVE, :NLIVE].T
    # e40jm: e40jm[32v+off+i, j*32+c] = E[i, j]
    blk = np.zeros((32, 320), np.float32)
    for off in (0, 16):
        for i in range(NLIVE):
            for j in range(NLIVE):
                blk[off + i, j * 32:(j + 1) * 32] = E[i, j]
    for v in range(4):
        cb16[32 * v:32 * v + 32, 256:576] = blk
    cb16 = cb16.astype(BF16)
    return cf8, cb16


def _run_device(hidden, W, b, transitions, trace=False, tmpdir=None):
    cf8, cb16 = _build_consts(W, b, transitions)
    flat = hidden.reshape(B * S, H)
    in_maps = []
    for core in range(NCORES):
        blk = flat[core * P_CORE:(core + 1) * P_CORE]        # [4096, 768]
        hT = np.ascontiguousarray(blk[PERM].T).astype(FP8NP)  # [768, 4096]
        in_maps.append({"hidT": hT, "cf8": cf8, "cb16": cb16})
    return run_bass_kernel_spmd(
        _get_nc(), in_maps, list(range(NCORES)), trace=trace, tmpdir=tmpdir)


def _logsumexp(x, axis):
    mx = np.max(x, axis=axis)
    mx_safe = np.where(np.isfinite(mx), mx, 0.0)
    out = mx + np.log(np.sum(np.exp(x - np.expand_dims(mx_safe, axis)), axis=axis))
    return np.where(np.isfinite(mx), out, -np.inf)


def _host_combine(results, transitions, tags):
    trans = transitions.astype(np.float64)
    err = np.errstate(invalid="ignore", divide="ignore", over="ignore")
    err.__enter__()

    # unpack fm_out: m and fsub in device (permuted) order -> original order
    feats = np.zeros((NCORES, P_CORE, T), np.float64)
    m_all = np.zeros((NCORES, P_CORE), np.float64)
    tt = np.arange(32)
    pp = np.arange(128)
    colidx = (tt[None, :] // 8) * 1024 + (tt[None, :] % 8) * 128 + pp[:, None]
    q_of = PERM[colidx]                # [p, t] original position
    for core, r in enumerate(results):
        fm = np.asarray(r["fm_out"]).astype(np.float64)   # [128, 416]
        m_dev = fm[:, 0:32]            # [p, t]
        fs_dev = fm[:, 32:416].reshape(128, 32, 12)   # [p, t, j]
        m_all[core, q_of] = m_dev
        feats[core, q_of, :] = fs_dev + m_dev[:, :, None]

    feats = feats.reshape(B, S, T)
    m_all = m_all.reshape(B, S)

    # unpack chunk matrices: A[b, k][i, j] (live 10x10), log + scale
    logA = np.zeros((B, KPG, NLIVE, NLIVE), np.float64)
    for core, r in enumerate(results):
        a = np.asarray(r["a_out"]).astype(np.float64)     # [128, 2560]
        a4 = a.reshape(128, NE, NLIVE, CPE)                # [p, E, j, c]
        for g in range(G):
            rows = 32 * (g // 2) + 16 * (g % 2)
            blkm = a4[rows:rows + NLIVE]                   # [i, E, j, c]
            logA[core * G + g] = np.log(blkm).transpose(1, 3, 0, 2).reshape(
                KPG, NLIVE, NLIVE)
    scale = m_all.reshape(B, KPG, L).sum(axis=2)           # [B, KPG]
    logA = logA + scale[:, :, None, None]

    # first global chunk: explicit recurrence from init (full 12-state)
    v0 = np.full(T, NEG, np.float64)
    v0[START] = 0.0
    for s in range(L):
        v0 = _logsumexp(trans[None, :, :] + v0[None, None, :], axis=2)[0] \
            + feats[0, s]
    ident = np.full((NLIVE, NLIVE), -np.inf)
    np.fill_diagonal(ident, 0.0)
    logA[0, 0] = ident

    # tree-combine the KPG chunk mats of each sentence -> one mat per sentence
    mats = logA.reshape(B * KPG, NLIVE, NLIVE)
    n = B * KPG
    while n > B:
        A2 = mats[0::2]        # earlier chunk
        B2 = mats[1::2]        # later chunk
        x = B2[:, :, :, None] + A2[:, None, :, :]
        mats = _logsumexp(x, axis=2)
        n //= 2

    # sequential carry across sentences
    last = np.zeros((B, T), np.float64)
    v = v0.copy()
    for bi in range(B):
        vl = _logsumexp(mats[bi] + v[None, :NLIVE], axis=1)
        v = np.concatenate([vl, [-np.inf, -np.inf]])
        last[bi] = v
    forward_score = _logsumexp(last + trans[STOP][None, :], axis=1)
    err.__exit__(None, None, None)

    tags = np.asarray(tags)
    tags_ext = np.concatenate(
        [np.full((B, 1), START, dtype=tags.dtype), tags], axis=1)
    prev, nxt = tags_ext[:, :-1], tags_ext[:, 1:]
    trans_sc = trans[nxt, prev].sum(axis=1)
    emit_sc = np.take_along_axis(
        feats, nxt[..., None].astype(np.int64), axis=2)[..., 0].sum(axis=1)
    gold = trans_sc + emit_sc + trans[STOP, tags_ext[:, -1]]
    gold_cum = np.cumsum(gold)
    out = np.sum(forward_score - gold_cum)
    return np.array([out], dtype=np.float32)


def kernel(hidden, W, b, transitions, tags, _trace=False, _tmpdir=None):
    hidden = np.asarray(hidden, dtype=np.float32)
    W = np.asarray(W, dtype=np.float32)
    b = np.asarray(b, dtype=np.float32)
    transitions = np.asarray(transitions, dtype=np.float32)
    res = _run_device(hidden, W, b, transitions, trace=_trace, tmpdir=_tmpdir)
    out = _host_combine(res.results, transitions, tags)
    if _trace:
        return out, res
    return out
